# revision 1
# baseline (speedup 1.0000x reference)
"""GAT layer kernel for Trainium2, 8 NeuronCores.

Problem: nn_GATLayer (B=4, N=2048, IN_F=256, OUT_F=64, H=8).

Key algebra: softmax over j of (src[b,i,h] + dst[b,j,h]) masked by adj[b,i,j].
src[b,i,h] is constant over j, so it cancels in the softmax:
    attn[b,i,j,h] = adj[b,i,j]*exp(dst[b,j,h]) / sum_j' adj[b,i,j']*exp(dst[b,j',h])
Therefore
    out[b,i,(h,f)] = (adj[b,i,:] @ g[b,:, (h,f)]) / (adj[b,i,:] @ e[b,:,h])
with hfeat = x@W (per-head features), dst[j,h] = x[j,:] @ (W . attn_dst)[:,h],
e = exp(dst), g = e * hfeat.  attn_src is mathematically irrelevant.

Sharding: 8 cores = 4 batches x 2 row-halves of i (softmax is over j only,
so row-sharding of i needs no communication).

Device dataflow per core (host does only layout prep of the sharded inputs):
  1. W / x^T / adj^T stream in via SWDGE cast-DMAs straight into fp32r
     operand tiles (int32 0/1 and fp32 cast in-flight; walrus accepts the
     cast-DMA as the fp32r rounding producer) -- no staging copies.
  2. hfeat = x@W and dst = x@(W.attn_dst) on PE; e = exp(dst) on ACT into
     g's denominator columns; g = e*hfeat as one DVE mul reading PSUM.
  3. numerator/denominator: per 128-row i-chunk, fp32r matmuls accumulated
     over 16 j-chunks, processed in two 4-chunk waves with group-major
     matmul order so PE consumes adj^T groups as their DMAs land.
  4. DVE reciprocal + broadcast multiply, per-chunk store DMAs.
"""

import numpy as np

B, N, IN_F, OUT_F, H = 4, 2048, 256, 64, 8
HF = H * OUT_F            # 512 concat features
NCORES = 8
ROWS = B * N // NCORES    # 1024 destination rows per core
P = 128
IC = ROWS // P            # 8 i-chunks per core
JC = N // P               # 16 j-chunks
KC = IN_F // P            # 2 k-chunks
JG = 8                    # adjT DMA groups (JC/JG j-chunks each)
WAVE = 4                  # i-chunks finalized per output store

# PE matmul dtype: "f32r" (1 cyc/row, relaxed precision) or "f32" (exact, 4 cyc/row)
MM_MODE = "f32r"

_CACHE = {}


def _bcast_last(ap, n):
    """View ap with an extra innermost broadcast (stride-0) dim of size n."""
    ap2 = ap.unsqueeze(len(ap.shape))
    return ap2.broadcast_to(tuple(ap.shape) + (n,))


def _build():
    import concourse.mybir as mybir
    import concourse.tile as tile
    from concourse import bacc

    f32 = mybir.dt.float32
    i32 = mybir.dt.int32
    MULT = mybir.AluOpType.mult
    mdt = mybir.dt.float32r if MM_MODE == "f32r" else f32

    nc = bacc.Bacc(trn_type="TRN2", debug=False, target_bir_lowering=False)

    # adjt[j, i] = adj[i, j] for this core's i-block (host layout prep)
    adjt_d = nc.dram_tensor("adjt", [N, ROWS], i32, kind="ExternalInput")
    w_d = nc.dram_tensor("w", [P, KC * HF], f32, kind="ExternalInput")
    adst_d = nc.dram_tensor("adst", [P, HF], f32, kind="ExternalInput")
    xt_d = nc.dram_tensor("xt", [P, 2 * KC * (N // 2)], f32, kind="ExternalInput")
    out_ds = [
        nc.dram_tensor(f"out{q}", [P, HF], f32, kind="ExternalOutput")
        for q in range(IC)
    ]

    with tile.TileContext(nc) as tc:
        with (
            tc.tile_pool(name="setup", bufs=1) as setup,
            tc.tile_pool(name="gpool", bufs=1) as gpool,
            tc.tile_pool(name="scratch", bufs=2) as scr,
            tc.tile_pool(name="adjT", bufs=1) as adjTp,
        ):
            # --- all input streams as cast-DMAs into fp32r tiles ---
            # W split by k-chunk and x^T by j-half so the projection's first
            # matmul waits on the smallest possible prefix of the stream
            w_sb = setup.tile([P, KC, HF], mdt)
            w_v = w_d.rearrange("p (kc n) -> p kc n", kc=KC)
            for kc in range(KC):
                nc.gpsimd.dma_start(w_sb[:, kc], w_v[:, kc])
            xT_sb = setup.tile([P, 2, KC, N // 2], mdt)
            xt_v = xt_d.rearrange("p (jh kc j) -> p jh kc j", jh=2, kc=KC)
            for jh in range(2):
                nc.gpsimd.dma_start(xT_sb[:, jh], xt_v[:, jh])
            adst_sb = setup.tile([P, HF], f32)
            nc.sync.dma_start(adst_sb[:], adst_d[:])
            # adj^T groups; j on partitions, no on-chip transpose
            nj = JC // JG
            adjT_g = []
            for G in range(JG):
                t = adjTp.tile([P, nj, ROWS], mdt, tag=f"adjt{G}")
                nc.gpsimd.dma_start(
                    t[:],
                    adjt_d[G * nj * P:(G + 1) * nj * P, :].rearrange(
                        "(jc jp) i -> jp jc i", jp=P),
                )
                adjT_g.append(t)

            def xT(kc, jc):
                jh, j0 = divmod(jc * P, N // 2)
                return xT_sb[:, jh, kc, j0:j0 + P]

            # wdst[kp, kc, h] = sum_f W[k, h*64+f] * attn_dst[h, f]
            # (folds attn_dst into the weight so dst = x @ wdst is a matmul)
            wdst_tmp = setup.tile([P, KC, H], f32)
            prod = scr.tile([P, KC, HF], f32)
            adst_b = adst_sb[:].unsqueeze(1).broadcast_to((P, KC, HF))
            nc.vector.tensor_tensor(prod[:], w_sb[:].bitcast(f32), adst_b, op=MULT)
            nc.vector.reduce_sum(
                out=wdst_tmp[:],
                in_=prod[:].rearrange("p kc (h f) -> p kc h f", h=H),
                axis=mybir.AxisListType.X,
            )
            wdst_sb = setup.tile([P, KC, H], mdt)
            nc.vector.tensor_copy(wdst_sb[:], wdst_tmp[:])

            # g_sb[jp, jc, 0:512]   = e * hfeat   (weighted features)
            # g_sb[jp, jc, 512:520] = e           (denominator columns)
            g_sb = gpool.tile([P, JC, HF + H], mdt)
            with (
                tc.tile_pool(name="ps_big", bufs=4, space="PSUM") as psbig,
                tc.tile_pool(name="ps_small", bufs=4, space="PSUM") as pssml,
            ):
                # ---- hfeat + e + g ----
                for jc in range(JC):
                    ph = psbig.tile([P, HF], f32, tag="big")
                    pd = pssml.tile([P, H], f32, tag="small")
                    for kc in range(KC):
                        nc.tensor.matmul(
                            ph[:], xT(kc, jc), w_sb[:, kc, :],
                            start=(kc == 0), stop=(kc == KC - 1),
                        )
                    for kc in range(KC):
                        nc.tensor.matmul(
                            pd[:], xT(kc, jc), wdst_sb[:, kc, :],
                            start=(kc == 0), stop=(kc == KC - 1),
                        )
                    e_cols = g_sb[:, jc, HF:HF + H]
                    nc.scalar.activation(
                        e_cols, pd[:], mybir.ActivationFunctionType.Exp
                    )
                    o3 = g_sb[:, jc, 0:HF].rearrange("p (h f) -> p h f", h=H)
                    h3 = ph[:].rearrange("p (h f) -> p h f", h=H)
                    e3 = _bcast_last(e_cols.bitcast(f32), OUT_F)
                    nc.vector.tensor_tensor(o3, h3, e3, op=MULT)

                # ---- main: waves of 4 i-chunks, j-group-major matmul order
                # so PE consumes adj^T groups as their DMAs land ----
                with tc.tile_pool(name="nsbp", bufs=4) as nsbp:
                    def finalize(ic, pF, pD):
                        rc = scr.tile([P, H], f32, tag="rc")
                        nc.vector.reciprocal(rc[:], pD[:])
                        nsb = nsbp.tile([P, HF], f32, tag="nsb")
                        n3 = nsb[:].rearrange("p (h f) -> p h f", h=H)
                        p3 = pF[:].rearrange("p (h f) -> p h f", h=H)
                        r3 = _bcast_last(rc[:], OUT_F)
                        nc.vector.tensor_tensor(n3, p3, r3, op=MULT)
                        nc.sync.dma_start(out_ds[ic][:, :], nsb[:])

                    def mm_pair(pF, pD, G, t, ic):
                        jc = G * nj + t
                        lhs = adjT_g[G][:, t, ic * P:(ic + 1) * P]
                        nc.tensor.matmul(
                            pF[:], lhs, g_sb[:, jc, 0:HF],
                            start=(jc == 0), stop=(jc == JC - 1),
                        )
                        nc.tensor.matmul(
                            pD[:], lhs, g_sb[:, jc, HF:HF + H],
                            start=(jc == 0), stop=(jc == JC - 1),
                        )

                    # wave 0 (ics 0-3): group-major -- arrival-paced, PE
                    # consumes adj^T groups as their DMAs land
                    pFs = [psbig.tile([P, HF], f32, tag="big",
                                      name=f"pF0_{k}") for k in range(WAVE)]
                    pDs = [pssml.tile([P, H], f32, tag="small",
                                      name=f"pD0_{k}") for k in range(WAVE)]
                    for G in range(JG):
                        for k in range(WAVE):
                            for t in range(nj):
                                mm_pair(pFs[k], pDs[k], G, t, k)
                    for k in range(WAVE):
                        finalize(k, pFs[k], pDs[k])

                    # wave 1 (ics 4-7): every adj^T group is resident by now,
                    # so run ic-major with inline finalize -- each chunk's
                    # store overlaps the next chunk's matmuls instead of all
                    # four stores firing at the kernel tail
                    for k in range(WAVE):
                        ic = WAVE + k
                        pF = psbig.tile([P, HF], f32, tag="big",
                                        name=f"pF1_{k}")
                        pD = pssml.tile([P, H], f32, tag="small",
                                        name=f"pD1_{k}")
                        for G in range(JG):
                            for t in range(nj):
                                mm_pair(pF, pD, G, t, ic)
                        finalize(ic, pF, pD)

    nc.compile()
    return nc


def _get_nc():
    if "nc" not in _CACHE:
        _CACHE["nc"] = _build()
    return _CACHE["nc"]


def _make_in_maps(x, adj, weight, attn_dst):
    x = np.ascontiguousarray(np.asarray(x), dtype=np.float32)
    adj = np.ascontiguousarray(np.asarray(adj), dtype=np.int32)
    weight = np.ascontiguousarray(np.asarray(weight), dtype=np.float32)
    attn_dst = np.ascontiguousarray(np.asarray(attn_dst), dtype=np.float32)

    adst_rep = np.ascontiguousarray(
        np.broadcast_to(attn_dst.reshape(1, HF), (P, HF)), dtype=np.float32)
    w_kp = np.ascontiguousarray(
        weight.reshape(KC, P, HF).transpose(1, 0, 2).reshape(P, KC * HF))

    in_maps = []
    for core in range(NCORES):
        b = core // 2
        half = core % 2
        # xt layout [p, jh, kc, j']: x[b][jh*1024 + j', kc*128 + p]
        xt = x[b].T.reshape(KC, P, 2, N // 2)          # [kc, p, jh, j']
        xt_kp = np.ascontiguousarray(
            xt.transpose(1, 2, 0, 3).reshape(P, 2 * KC * (N // 2)))
        adjt = adj[b].T[:, half * ROWS:(half + 1) * ROWS]  # [N, ROWS]
        in_maps.append({
            "adjt": np.ascontiguousarray(adjt),
            "w": w_kp,
            "adst": adst_rep,
            "xt": xt_kp,
        })
    return in_maps


def _run_device(in_maps):
    from concourse import bass_utils

    nc = _get_nc()
    res = bass_utils.run_bass_kernel_spmd(
        nc, in_maps, core_ids=list(range(NCORES)))
    return [dict(r) for r in res.results]


def _run_device_subprocess(in_maps):
    """Fresh-process fallback: a wedged accelerator surfaces as
    NRT_EXEC_UNIT_UNRECOVERABLE and poisons the in-process PJRT client;
    a new process gets a fresh axon session and a reset device."""
    import os
    import pickle
    import subprocess
    import sys
    import tempfile

    d = tempfile.mkdtemp(prefix="gat_kernel_")
    inp = os.path.join(d, "in.pkl")
    outp = os.path.join(d, "out.pkl")
    with open(inp, "wb") as f:
        pickle.dump(in_maps, f)
    code = (
        "import pickle, sys\n"
        f"sys.path.insert(0, {os.path.dirname(os.path.abspath(__file__))!r})\n"
        "import kernel\n"
        f"in_maps = pickle.load(open({inp!r}, 'rb'))\n"
        f"pickle.dump(kernel._run_device(in_maps), open({outp!r}, 'wb'))\n"
    )
    env = dict(os.environ, GAT_KERNEL_SUBPROC="1")
    subprocess.run([sys.executable, "-c", code], check=True, env=env,
                   timeout=1200)
    with open(outp, "rb") as f:
        return pickle.load(f)


def kernel(x, adj, weight, attn_src, attn_dst):
    import os
    import time

    in_maps = _make_in_maps(x, adj, weight, attn_dst)
    try:
        results = _run_device(in_maps)
    except Exception:
        if os.environ.get("GAT_KERNEL_SUBPROC") == "1":
            raise
        time.sleep(2)
        results = _run_device_subprocess(in_maps)

    out = np.empty((B, N, HF), dtype=np.float32)
    for core in range(NCORES):
        b = core // 2
        half = core % 2
        for q in range(IC):
            r0 = half * ROWS + q * P
            out[b, r0:r0 + P, :] = results[core][f"out{q}"]
    return out



# revision 2
# speedup vs baseline: 1.0608x; 1.0608x over previous
"""GAT layer kernel for Trainium2, 8 NeuronCores.

Problem: nn_GATLayer (B=4, N=2048, IN_F=256, OUT_F=64, H=8).

Key algebra: softmax over j of (src[b,i,h] + dst[b,j,h]) masked by adj[b,i,j].
src[b,i,h] is constant over j, so it cancels in the softmax:
    attn[b,i,j,h] = adj[b,i,j]*exp(dst[b,j,h]) / sum_j' adj[b,i,j']*exp(dst[b,j',h])
Therefore
    out[b,i,(h,f)] = (adj[b,i,:] @ g[b,:, (h,f)]) / (adj[b,i,:] @ e[b,:,h])
with hfeat = x@W (per-head features), dst[j,h] = x[j,:] @ (W . attn_dst)[:,h],
e = exp(dst), g = e * hfeat.  attn_src is mathematically irrelevant.

Sharding: 8 cores = 4 batches x 2 row-halves of i (softmax is over j only,
so row-sharding of i needs no communication).

v2: all-bf16 datapath. Host casts x/W/adj to bf16 and folds attn_dst into
the weight (wdst = sum_f W[.,h*64+f]*attn_dst[h,f], a tiny weight-prep
contraction).  Device: bf16 projection matmuls -> f32 psum, ACT exp,
DVE g-multiply (bf16), bf16 aggregation matmuls, DVE reciprocal+scale,
bf16 stores upcast to f32 on the host.  All input DMA bytes halved vs
the f32 baseline; adj (the dominant stream) is 4MB/core instead of 8MB.
"""

import numpy as np
import ml_dtypes

B, N, IN_F, OUT_F, H = 4, 2048, 256, 64, 8
HF = H * OUT_F            # 512 concat features
NCORES = 8
ROWS = B * N // NCORES    # 1024 destination rows per core
P = 128
IC = ROWS // P            # 8 i-chunks per core
JC = N // P               # 16 j-chunks
KC = IN_F // P            # 2 k-chunks
JG = 8                    # adjT DMA groups (JC/JG j-chunks each)
WAVE = 4                  # i-chunks in the first (arrival-paced) wave

BF16 = ml_dtypes.bfloat16

_CACHE = {}


def _bcast_last(ap, n):
    """View ap with an extra innermost broadcast (stride-0) dim of size n."""
    ap2 = ap.unsqueeze(len(ap.shape))
    return ap2.broadcast_to(tuple(ap.shape) + (n,))


def _build():
    import concourse.mybir as mybir
    import concourse.tile as tile
    from concourse import bacc

    f32 = mybir.dt.float32
    bf = mybir.dt.bfloat16
    MULT = mybir.AluOpType.mult

    nc = bacc.Bacc(trn_type="TRN2", debug=False, target_bir_lowering=False)

    # adjt[j, i] = adj[i, j] for this core's i-block (host layout prep)
    adjt_d = nc.dram_tensor("adjt", [N, ROWS], bf, kind="ExternalInput")
    w_d = nc.dram_tensor("w", [P, KC * HF], bf, kind="ExternalInput")
    wdst_d = nc.dram_tensor("wdst", [P, KC * H], bf, kind="ExternalInput")
    xt_d = nc.dram_tensor("xt", [P, 2 * KC * (N // 2)], bf, kind="ExternalInput")
    out_ds = [
        nc.dram_tensor(f"out{q}", [P, HF], bf, kind="ExternalOutput")
        for q in range(IC)
    ]

    with tile.TileContext(nc) as tc:
        with (
            tc.tile_pool(name="setup", bufs=1) as setup,
            tc.tile_pool(name="gpool", bufs=1) as gpool,
            tc.tile_pool(name="scratch", bufs=2) as scr,
            tc.tile_pool(name="adjT", bufs=1) as adjTp,
        ):
            # --- input streams ---
            # W / wdst first (small), then x^T by j-half, then adj^T groups
            w_sb = setup.tile([P, KC, HF], bf)
            w_v = w_d.rearrange("p (kc n) -> p kc n", kc=KC)
            for kc in range(KC):
                nc.gpsimd.dma_start(w_sb[:, kc], w_v[:, kc])
            wdst_sb = setup.tile([P, KC, H], bf)
            nc.sync.dma_start(
                wdst_sb[:], wdst_d.rearrange("p (kc h) -> p kc h", kc=KC))
            xT_sb = setup.tile([P, 2, KC, N // 2], bf)
            xt_v = xt_d.rearrange("p (jh kc j) -> p jh kc j", jh=2, kc=KC)
            for jh in range(2):
                nc.gpsimd.dma_start(xT_sb[:, jh], xt_v[:, jh])
            # adj^T groups; j on partitions, no on-chip transpose
            nj = JC // JG
            adjT_g = []
            for G in range(JG):
                t = adjTp.tile([P, nj, ROWS], bf, tag=f"adjt{G}")
                nc.gpsimd.dma_start(
                    t[:],
                    adjt_d[G * nj * P:(G + 1) * nj * P, :].rearrange(
                        "(jc jp) i -> jp jc i", jp=P),
                )
                adjT_g.append(t)

            def xT(kc, jc):
                jh, j0 = divmod(jc * P, N // 2)
                return xT_sb[:, jh, kc, j0:j0 + P]

            # g_sb[jp, jc, :] = e * hfeat   (weighted features, bf16)
            # e_sb[jp, jc, :] = e = exp(dst)  (bf16)
            g_sb = gpool.tile([P, JC, HF], bf)
            e_sb = gpool.tile([P, JC, H], bf)
            with (
                tc.tile_pool(name="ps_big", bufs=4, space="PSUM") as psbig,
                tc.tile_pool(name="ps_small", bufs=4, space="PSUM") as pssml,
            ):
                # ---- hfeat + dst + e + g ----
                for jc in range(JC):
                    ph = psbig.tile([P, HF], f32, tag="big")
                    pd = pssml.tile([P, H], f32, tag="small")
                    for kc in range(KC):
                        nc.tensor.matmul(
                            ph[:], xT(kc, jc), w_sb[:, kc, :],
                            start=(kc == 0), stop=(kc == KC - 1),
                        )
                    for kc in range(KC):
                        nc.tensor.matmul(
                            pd[:], xT(kc, jc), wdst_sb[:, kc, :],
                            start=(kc == 0), stop=(kc == KC - 1),
                        )
                    e_cols = e_sb[:, jc, :]
                    nc.scalar.activation(
                        e_cols, pd[:], mybir.ActivationFunctionType.Exp
                    )
                    o3 = g_sb[:, jc, :].rearrange("p (h f) -> p h f", h=H)
                    h3 = ph[:].rearrange("p (h f) -> p h f", h=H)
                    e3 = _bcast_last(e_cols, OUT_F)
                    nc.vector.tensor_tensor(o3, h3, e3, op=MULT)

                # ---- main: aggregation matmuls, two waves of 4 i-chunks ----
                with tc.tile_pool(name="nsbp", bufs=4) as nsbp:
                    def finalize(ic, pF, pD):
                        rc = scr.tile([P, H], f32, tag="rc")
                        nc.vector.reciprocal(rc[:], pD[:])
                        nsb = nsbp.tile([P, HF], bf, tag="nsb")
                        n3 = nsb[:].rearrange("p (h f) -> p h f", h=H)
                        p3 = pF[:].rearrange("p (h f) -> p h f", h=H)
                        r3 = _bcast_last(rc[:], OUT_F)
                        nc.vector.tensor_tensor(n3, p3, r3, op=MULT)
                        nc.sync.dma_start(out_ds[ic][:, :], nsb[:])

                    def mm_pair(pF, pD, jc, ic):
                        G, t = divmod(jc, nj)
                        lhs = adjT_g[G][:, t, ic * P:(ic + 1) * P]
                        nc.tensor.matmul(
                            pF[:], lhs, g_sb[:, jc, :],
                            start=(jc == 0), stop=(jc == JC - 1),
                        )
                        nc.tensor.matmul(
                            pD[:], lhs, e_sb[:, jc, :],
                            start=(jc == 0), stop=(jc == JC - 1),
                        )

                    # wave 0 (ics 0-3): jc-major -- arrival-paced, PE
                    # consumes adj^T groups / g chunks as they land
                    pFs = [psbig.tile([P, HF], f32, tag="big",
                                      name=f"pF0_{k}") for k in range(WAVE)]
                    pDs = [pssml.tile([P, H], f32, tag="small",
                                      name=f"pD0_{k}") for k in range(WAVE)]
                    for jc in range(JC):
                        for k in range(WAVE):
                            mm_pair(pFs[k], pDs[k], jc, k)
                    for k in range(WAVE):
                        finalize(k, pFs[k], pDs[k])

                    # wave 1 (ics 4-7): everything resident; ic-major with
                    # inline finalize so stores overlap the next chunk
                    for k in range(WAVE):
                        ic = WAVE + k
                        pF = psbig.tile([P, HF], f32, tag="big",
                                        name=f"pF1_{k}")
                        pD = pssml.tile([P, H], f32, tag="small",
                                        name=f"pD1_{k}")
                        for jc in range(JC):
                            mm_pair(pF, pD, jc, ic)
                        finalize(ic, pF, pD)

    nc.compile()
    return nc


def _get_nc():
    if "nc" not in _CACHE:
        _CACHE["nc"] = _build()
    return _CACHE["nc"]


def _make_in_maps(x, adj, weight, attn_dst):
    x = np.ascontiguousarray(np.asarray(x), dtype=np.float32)
    adj = np.asarray(adj)
    weight = np.ascontiguousarray(np.asarray(weight), dtype=np.float32)
    attn_dst = np.ascontiguousarray(np.asarray(attn_dst), dtype=np.float32)

    # fold attn_dst into the weight: wdst[k, h] = sum_f W[k, h*64+f]*adst[h, f]
    wdst = (weight.reshape(IN_F, H, OUT_F) * attn_dst[None]).sum(-1)

    w_kp = np.ascontiguousarray(
        weight.reshape(KC, P, HF).transpose(1, 0, 2).reshape(P, KC * HF)
    ).astype(BF16)
    wdst_kp = np.ascontiguousarray(
        wdst.reshape(KC, P, H).transpose(1, 0, 2).reshape(P, KC * H)
    ).astype(BF16)

    in_maps = []
    for core in range(NCORES):
        b = core // 2
        half = core % 2
        # xt layout [p, jh, kc, j']: x[b][jh*1024 + j', kc*128 + p]
        xt = x[b].T.reshape(KC, P, 2, N // 2)          # [kc, p, jh, j']
        xt_kp = np.ascontiguousarray(
            xt.transpose(1, 2, 0, 3).reshape(P, 2 * KC * (N // 2))
        ).astype(BF16)
        adjt = adj[b].T[:, half * ROWS:(half + 1) * ROWS]  # [N, ROWS]
        in_maps.append({
            "adjt": np.ascontiguousarray(adjt, dtype=np.float32).astype(BF16),
            "w": w_kp,
            "wdst": wdst_kp,
            "xt": xt_kp,
        })
    return in_maps


def _run_device(in_maps):
    from concourse import bass_utils

    nc = _get_nc()
    res = bass_utils.run_bass_kernel_spmd(
        nc, in_maps, core_ids=list(range(NCORES)))
    return [dict(r) for r in res.results]


def _run_device_subprocess(in_maps):
    """Fresh-process fallback: a wedged accelerator surfaces as
    NRT_EXEC_UNIT_UNRECOVERABLE and poisons the in-process PJRT client;
    a new process gets a fresh axon session and a reset device."""
    import os
    import pickle
    import subprocess
    import sys
    import tempfile

    d = tempfile.mkdtemp(prefix="gat_kernel_")
    inp = os.path.join(d, "in.pkl")
    outp = os.path.join(d, "out.pkl")
    with open(inp, "wb") as f:
        pickle.dump(in_maps, f)
    code = (
        "import pickle, sys\n"
        f"sys.path.insert(0, {os.path.dirname(os.path.abspath(__file__))!r})\n"
        "import kernel\n"
        f"in_maps = pickle.load(open({inp!r}, 'rb'))\n"
        f"pickle.dump(kernel._run_device(in_maps), open({outp!r}, 'wb'))\n"
    )
    env = dict(os.environ, GAT_KERNEL_SUBPROC="1")
    subprocess.run([sys.executable, "-c", code], check=True, env=env,
                   timeout=1200)
    with open(outp, "rb") as f:
        return pickle.load(f)


def kernel(x, adj, weight, attn_src, attn_dst):
    import os
    import time

    in_maps = _make_in_maps(x, adj, weight, attn_dst)
    try:
        results = _run_device(in_maps)
    except Exception:
        if os.environ.get("GAT_KERNEL_SUBPROC") == "1":
            raise
        time.sleep(2)
        results = _run_device_subprocess(in_maps)

    out = np.empty((B, N, HF), dtype=np.float32)
    for core in range(NCORES):
        b = core // 2
        half = core % 2
        for q in range(IC):
            r0 = half * ROWS + q * P
            out[b, r0:r0 + P, :] = results[core][f"out{q}"].astype(np.float32)
    return out
